# revision 18
# baseline (speedup 1.0000x reference)
"""Trainium2 Bass kernel for nn_CodeformerLM (masked embedding -> W_dec -> logits).

The reference computation reduces to:
    mask[b,c,t] = (t < split_sizes[b,c]) & (c < num_chunks[b]),  t in [0, T-2]
    X = word_embeddings[token_ids_chunk[:, :, :T-1]] * mask      # [B,C,T-1,H]
    logits = (X @ W_dec) @ word_embeddings.T                     # [B,C,T-1,V]
(the gathered decoder positions c+1+t never touch the chunk_units/SOS prefix,
and PAD_VAL == 0, so chunk_units / chunk_sos_embedding cannot affect the output)

Sharding: vocab (tensor-parallel) across the 8 cores; every core processes the
compacted active rows (host masks + gathers the embedding rows, so the device
receives dense transposed operand tiles).

All matmuls run in fp8 e4m3 with the PE DoubleRow perf mode (2 contraction
chunks per instruction at 0.5 cycles/output-column). Accuracy is recovered
with a hi/lo split: A ~= Ah + Al, Ah = e4m3(A*s), Al = e4m3(A*s - Ah) (the
residual lands in lower e4m3 binades, no extra scale needed), and
  A@B ~= Ah@Bh + Al@Bh + Ah@Bl          (error ~ Al@Bl ~ 0.1%)
at 0.75x the bf16 cycle count. All scales are powers of two; the final
descale happens on the host (exact).

With NSLOT=8 the (uh,el) correction for the last kc-pair is dropped:
measured rel err 0.0136 vs the 2e-2 gate, for 8/9 of the phase-3 cycles and
a 5/6 E-stream.

Per-core device pipeline:
  1. DMA in (SP/HWDGE queue): xh, wh, wl (two mc-halves so the hl pass
     chases the stream), xl, then the E^T shard in (quarter x kc-pair)
     stripe-pair pieces, hi/lo interleaved in slot-consumption order.
  2. Phase 2: U^T = W_dec^T X^T in 6 PSUM tiles: pass hh (kp-outer), hl
     (mc-outer, chasing wl), then lh mc-by-mc with stop; per-mc quantize
     Uh = e4m3(psum*d) on the Act engine, Ul = e4m3(psum*d - Uh) on the DVE
     (fused scalar_tensor_tensor).
  3. Phase 3: logits^T shard = U E^T; per vocab quarter two waves of
     (3 row-tile) PSUM groups, slot-layer-major so the PE chases the E
     stream; PSUM -> SBUF f16 copies alternate DVE/Act; the last quarter
     runs wave-interleaved group-serial with out pieces alternating the
     SP/Act DGE queues (the shared HWDGE drains them in real time) and ends
     on a small 256-column piece to shrink the post-compute drain.

All matmul operand strides stay multiples of 128 (fp8 dual-row Ldweights
fails walrus's s3_lw_dual_fp8_restrictions ISA check otherwise); row
trimming to the active count happens only in the out DMAs.
"""

import numpy as np
import ml_dtypes

B, C, T = 4, 16, 33
TT = T - 1            # 32 token positions actually used
H = 768
HC = H // 128         # 6 contraction chunks
KP = HC // 2          # 3 DoubleRow kc-pairs
V = 32000
NCORES = 8
VS = V // NCORES      # 4000 vocab columns per core
VT = 500              # vocab stripe (one PSUM bank holds 512 f32)
NST = VS // VT        # 8 stripes per core
VQ = 1000             # vocab quarter (2 stripes = PSUM wave pair)
NQ = VS // VQ         # 4 quarters
F8 = ml_dtypes.float8_e4m3
NWARM = 9             # PE clock-ramp warmup matmuls
NSLOT = 8             # 9 = full hi/lo (3 terms x 3 kc-pairs); 8 drops (uh,el,kp2)

_KERNELS = {}
last_results = None   # BassKernelResults of the most recent run (for test harness)


def _build(n: int, d: float):
    """Build + compile the 8-core SPMD bass kernel for n active rows."""
    import concourse.bacc as bacc
    import concourse.bass as bass
    import concourse.mybir as mybir
    import concourse.tile as tile

    dt = mybir.dt
    DR = mybir.MatmulPerfMode.DoubleRow
    MT = (n + 127) // 128
    npad = MT * 128           # fp8 dual ldweights reject non-128-mult strides
    drows = [min(128, n - 128 * mt) for mt in range(MT)]
    nc = bacc.Bacc("TRN2", target_bir_lowering=False, debug=False,
                   num_devices=NCORES)

    xh_d = nc.dram_tensor("xh", [128, HC, npad], dt.float8e4, kind="ExternalInput")
    xl_d = nc.dram_tensor("xl", [128, HC, npad], dt.float8e4, kind="ExternalInput")
    wh_d = nc.dram_tensor("wh", [128, HC, H], dt.float8e4, kind="ExternalInput")
    # wl in mc-major layout: wl[p, mc, kc, j] = Wl[kc*128+p, mc*128+j]
    wl_d = nc.dram_tensor("wl", [128, HC, HC, 128], dt.float8e4,
                          kind="ExternalInput")
    # E^T shard: eh[p, kc, s, j] = Eh[s*VT+j, kc*128+p]; el holds the lo
    # residual only for the kc-pairs that get the (uh,el) correction term
    KPL = KP if NSLOT == 9 else KP - 1
    eh_d = nc.dram_tensor("ehi", [128, HC, NST, VT], dt.float8e4,
                          kind="ExternalInput")
    el_d = nc.dram_tensor("elo", [128, 2 * KPL, NST, VT], dt.float8e4,
                          kind="ExternalInput")
    # f16 output, scaled by S = se*sw*d*se; host descales (exact, powers of 2)
    out = nc.dram_tensor("out", [n, VS], dt.float16, kind="ExternalOutput")

    with tile.TileContext(nc) as tc:
        with (
            tc.tile_pool(name="const", bufs=1) as cpool,
            tc.tile_pool(name="outb", bufs=6) as opool,
            tc.tile_pool(name="ps", bufs=8, space=bass.MemorySpace.PSUM) as pspool,
        ):
            # PE warmup: the clock-ramp model holds the PE below 2.4 GHz
            # until it has been busy ~3 us; the first ~4 us are input DMA, so
            # burn that window on junk matmuls.
            warm_sb = cpool.tile([128, 512], dt.bfloat16, tag="warm", name="warm_sb")
            nc.vector.memset(warm_sb[:], 0.0)
            pw = pspool.tile([128, 512], dt.float32, tag="ps", name="pw")
            for _ in range(NWARM):
                nc.tensor.matmul(pw[:], warm_sb[:, :128], warm_sb[:],
                                 start=True, stop=True)

            # ---- input DMAs (SP queue, program order = service order) ----
            xh_sb = cpool.tile([128, HC, npad], dt.float8e4, tag="xh", name="xh_sb")
            xl_sb = cpool.tile([128, HC, npad], dt.float8e4, tag="xl", name="xl_sb")
            wh_sb = cpool.tile([128, HC, H], dt.float8e4, tag="wh", name="wh_sb")
            wl_sb = cpool.tile([128, HC, HC, 128], dt.float8e4, tag="wl",
                               name="wl_sb")
            nc.sync.dma_start(xh_sb[:], xh_d.ap()[:])
            nc.sync.dma_start(wh_sb[:], wh_d.ap()[:])
            # wl in two mc-halves: the hl pass starts on the first half while
            # the second streams (smaller pieces would be HWDGE-cadence-bound)
            nc.sync.dma_start(wl_sb[:, :HC // 2, :, :],
                              wl_d.ap()[:, :HC // 2, :, :])
            nc.sync.dma_start(wl_sb[:, HC // 2:, :, :],
                              wl_d.ap()[:, HC // 2:, :, :])
            nc.sync.dma_start(xl_sb[:], xl_d.ap()[:])

            # E^T stream in (quarter x kc-pair) stripe-pair pieces,
            # hi/lo interleaved in phase-3 slot consumption order
            eh_sb = cpool.tile([128, HC, NST, VT], dt.float8e4, tag="ehi",
                               name="eh_sb")
            el_sb = cpool.tile([128, 2 * KPL, NST, VT], dt.float8e4, tag="elo",
                               name="el_sb")
            for q in range(NQ):
                s0 = 2 * q
                for kp in range(KP):
                    k0, k1 = 2 * kp, 2 * kp + 2
                    nc.sync.dma_start(eh_sb[:, k0:k1, s0:s0 + 2, :],
                                      eh_d.ap()[:, k0:k1, s0:s0 + 2, :])
                    if kp < KPL:
                        nc.sync.dma_start(el_sb[:, k0:k1, s0:s0 + 2, :],
                                          el_d.ap()[:, k0:k1, s0:s0 + 2, :])

            # ---- phase 2: U accumulation (3 DoubleRow passes) ----
            psu = [pspool.tile([128, npad], dt.float32, tag="ps",
                               name=f"psu{mc}", padded_shape=[128, 512])
                   for mc in range(HC)]
            first = [True] * HC
            for kp in range(KP):            # pass hh, kp-outer to chase wh
                for mc in range(HC):
                    nc.tensor.matmul(
                        psu[mc][:],
                        wh_sb[:, 2 * kp:2 * kp + 2, mc * 128:(mc + 1) * 128],
                        xh_sb[:, 2 * kp:2 * kp + 2, :],
                        start=first[mc], stop=False, perf_mode=DR)
                    first[mc] = False
            for mc in range(HC):            # pass hl, mc-outer to chase wl
                for kp in range(KP):
                    nc.tensor.matmul(
                        psu[mc][:],
                        wl_sb[:, mc, 2 * kp:2 * kp + 2, :],
                        xh_sb[:, 2 * kp:2 * kp + 2, :],
                        start=False, stop=False, perf_mode=DR)
            uh = cpool.tile([128, HC, npad], dt.float8e4, tag="uh", name="uh_sb")
            ul = cpool.tile([128, HC, npad], dt.float8e4, tag="ul", name="ul_sb")
            for mc in range(HC):            # pass lh, mc-outer: stop + quantize
                for kp in range(KP):
                    nc.tensor.matmul(
                        psu[mc][:],
                        wh_sb[:, 2 * kp:2 * kp + 2, mc * 128:(mc + 1) * 128],
                        xl_sb[:, 2 * kp:2 * kp + 2, :],
                        start=False, stop=(kp == KP - 1), perf_mode=DR)
                nc.scalar.mul(uh[:, mc, :], psu[mc][:], float(d))
                nc.vector.scalar_tensor_tensor(
                    ul[:, mc, :], psu[mc][:], float(d), uh[:, mc, :],
                    op0=mybir.AluOpType.mult, op1=mybir.AluOpType.subtract)

            # ---- phase 3: logits shard ----
            # slots kc-pair-major: (uh,eh) (ul,eh) (uh,el) per kp
            slots = []
            for kp in range(KP):
                for term in range(3):
                    slots.append((kp, term))
            if NSLOT == 8:
                slots = [s for s in slots if s != (KP - 1, 2)]
            ncopy = 0

            def do_group(q, nt, mt, c0, clen, psl_t, obs, s_dma):
                nonlocal ncopy
                st = 2 * q + nt
                for s, (kp, term) in enumerate(slots):
                    k0, k1 = 2 * kp, 2 * kp + 2
                    usb = ul if term == 1 else uh
                    esb = el_sb if term == 2 else eh_sb
                    nc.tensor.matmul(
                        psl_t[:],
                        usb[:, k0:k1, mt * 128:(mt + 1) * 128],
                        esb[:, k0:k1, st, c0:c0 + clen],
                        start=(s == 0), stop=(s == NSLOT - 1),
                        perf_mode=DR)
                dst = obs[mt][:, nt * VT + c0:nt * VT + c0 + clen]
                if ncopy % 2 == 0:
                    nc.vector.tensor_copy(dst, psl_t[:])
                else:
                    nc.scalar.copy(dst, psl_t[:])
                ncopy += 1
                if s_dma:
                    rows = drows[mt]
                    eng = (nc.sync, nc.scalar)[ncopy % 2]
                    eng.dma_start(
                        out.ap()[mt * 128:mt * 128 + rows,
                                 st * VT + c0:st * VT + c0 + clen],
                        obs[mt][:rows, nt * VT + c0:nt * VT + c0 + clen])

            for q in range(NQ - 1):
                obs = [opool.tile([128, VQ], dt.float16, tag="outb",
                                  name=f"ob{q}_{mt}") for mt in range(MT)]
                for nt in range(2):
                    st = 2 * q + nt
                    psl = [pspool.tile([128, VT], dt.float32, tag="ps",
                                       name=f"psl{st}_{mt}",
                                       padded_shape=[128, 512])
                           for mt in range(MT)]
                    # slot-layer-major chases the E stream
                    for s in range(NSLOT):
                        kp, term = slots[s]
                        k0, k1 = 2 * kp, 2 * kp + 2
                        for mt in range(MT):
                            usb = ul if term == 1 else uh
                            esb = el_sb if term == 2 else eh_sb
                            nc.tensor.matmul(
                                psl[mt][:],
                                usb[:, k0:k1, mt * 128:(mt + 1) * 128],
                                esb[:, k0:k1, st, :],
                                start=(s == 0), stop=(s == NSLOT - 1),
                                perf_mode=DR)
                            if s == NSLOT - 1:
                                dst = obs[mt][:, nt * VT:(nt + 1) * VT]
                                if ncopy % 2 == 0:
                                    nc.vector.tensor_copy(dst, psl[mt][:])
                                else:
                                    nc.scalar.copy(dst, psl[mt][:])
                                ncopy += 1
                                if nt == 1:
                                    rows = drows[mt]
                                    nc.sync.dma_start(
                                        out.ap()[mt * 128:mt * 128 + rows,
                                                 q * VQ:(q + 1) * VQ],
                                        obs[mt][:rows, :])

            # last quarter: wave-interleaved group-serial jobs so the stops
            # (and their copy+DMA chains) spread at ~0.85 us — faster than
            # the shared HWDGE drains them — and the drain ends on a small
            # 256-column piece
            q = NQ - 1
            obs = [opool.tile([128, VQ], dt.float16, tag="outb",
                              name=f"ob{q}_{mt}") for mt in range(MT)]
            jobs = []
            for mt in range(MT):
                if mt == MT - 1:
                    jobs += [(0, mt, 0, VT), (1, mt, 0, 244), (1, mt, 244, 256)]
                else:
                    jobs += [(0, mt, 0, VT), (1, mt, 0, VT)]
            for ji, (nt, mt, c0, clen) in enumerate(jobs):
                psl_t = pspool.tile([128, clen], dt.float32, tag="ps",
                                    name=f"pslq3_{ji}", padded_shape=[128, 512])
                do_group(q, nt, mt, c0, clen, psl_t, obs, True)

    nc.compile()
    return nc


def _get_kernel(n: int, d: float):
    key = (n, float(d))
    if key not in _KERNELS:
        _KERNELS[key] = _build(n, d)
    return _KERNELS[key]


def _pow2floor(x):
    return float(2.0 ** np.floor(np.log2(x)))


def prep_inputs(token_ids, split_sizes, num_chunks, E, Wd):
    """Host-side shard prep. Returns (in_maps, rows, nact, S, d)."""
    b, c, t = token_ids.shape
    tt = t - 1
    mask = ((np.arange(tt)[None, None, :] < split_sizes[:, :, None])
            & (np.arange(c)[None, :, None] < num_chunks[:, None, None]))
    flat_ids = token_ids[:, :, :tt].reshape(-1).astype(np.int64)
    rows = np.nonzero(mask.reshape(-1))[0]
    nact = len(rows)
    if nact == 0:
        return None, rows, 0, 1.0, 1.0
    ids = flat_ids[rows]

    f32 = np.float32
    E = np.ascontiguousarray(E, dtype=f32)
    Wd = np.ascontiguousarray(Wd, dtype=f32)

    # power-of-two scales: hi parts land in (96, 192], residuals fall into
    # lower e4m3 binades naturally
    se = _pow2floor(192.0 / float(np.abs(E).max()))
    sw = _pow2floor(192.0 / float(np.abs(Wd).max()))
    Eh = (E * se).astype(F8)
    El = ((E * se) - Eh.astype(f32)).astype(F8)
    Wq = Wd * sw
    Wh = Wq.astype(F8)
    Wl = (Wq - Wh.astype(f32)).astype(F8)

    # U = X @ W scale d: bound max|U| <= max||X_i|| * sigma_max(W) (power
    # iteration), and bound the f16 output range via max||E_j||.
    Xrows = E[ids]
    maxXn = float(np.sqrt((Xrows * Xrows).sum(axis=1)).max())
    v = np.ones(H, dtype=f32) / np.sqrt(H)
    for _ in range(8):
        w = Wd @ v
        v = Wd.T @ w
        v /= float(np.linalg.norm(v))
    sigW = float(np.sqrt(np.linalg.norm(Wd.T @ (Wd @ v))))
    Ubound = max(maxXn * sigW, 1e-30)
    Erown = float(np.sqrt((E * E).sum(axis=1)).max())
    Lbound = Ubound * Erown
    d = min(_pow2floor(192.0 / (Ubound * se * sw)),
            _pow2floor(30000.0 / (Lbound * se * sw * se)))
    S = se * sw * d * se

    # transposed layouts: [128, HC, n] with partitions = H-chunk lanes
    def t_rows(A, n):
        return np.ascontiguousarray(A.reshape(n, HC, 128).transpose(2, 1, 0))

    npad = ((nact + 127) // 128) * 128
    Xq = np.zeros((npad, H), dtype=F8)
    Xq[:nact] = Eh[ids]
    xh_np = t_rows(Xq, npad)
    Xq[:nact] = El[ids]
    xl_np = t_rows(Xq, npad)
    wh_np = np.ascontiguousarray(Wh.reshape(HC, 128, H).transpose(1, 0, 2))
    # wl mc-major: [128, mc, kc, 128]
    wl_np = np.ascontiguousarray(
        Wl.reshape(HC, 128, HC, 128).transpose(1, 2, 0, 3))

    KPL = KP if NSLOT == 9 else KP - 1
    in_maps = []
    for k in range(NCORES):
        sl = slice(k * VS, (k + 1) * VS)
        ehT = t_rows(Eh[sl], VS).reshape(128, HC, NST, VT)
        elT = np.ascontiguousarray(
            t_rows(El[sl], VS).reshape(128, HC, NST, VT)[:, :2 * KPL])
        in_maps.append({"xh": xh_np, "xl": xl_np, "wh": wh_np, "wl": wl_np,
                        "ehi": ehT, "elo": elT})
    return in_maps, rows, nact, S, d


def kernel(**inputs) -> np.ndarray:
    global last_results
    token_ids = np.asarray(inputs["token_ids_chunk"])
    split_sizes = np.asarray(inputs["split_sizes"])
    num_chunks = np.asarray(inputs["num_chunks"])
    E = np.asarray(inputs["word_embeddings"], dtype=np.float32)
    Wd = np.asarray(inputs["W_dec"], dtype=np.float32)
    # chunk_units / chunk_sos_embedding provably do not affect the output.

    b, c, t = token_ids.shape
    tt = t - 1
    outF = np.zeros((b * c * tt, V), dtype=np.float32)

    in_maps, rows, nact, S, d = prep_inputs(
        token_ids, split_sizes, num_chunks, E, Wd)
    if in_maps is not None:
        import time
        from concourse import bass_utils
        nc = _get_kernel(nact, d)
        res = None
        for attempt in range(4):
            try:
                res = bass_utils.run_bass_kernel_spmd(
                    nc, in_maps, core_ids=list(range(NCORES)))
                break
            except Exception:
                # the tunneled device occasionally reports a transient
                # NRT_EXEC_UNIT_UNRECOVERABLE; retry (with a core reset
                # requested) usually clears it
                if attempt == 3:
                    raise
                import os
                os.environ["NEURON_RT_RESET_CORES"] = "1"
                time.sleep(10)
        last_results = res
        shard = np.concatenate(
            [res.results[k]["out"].astype(np.float32)
             for k in range(NCORES)], axis=1) * np.float32(1.0 / S)
        outF[rows] = shard
    return outF.reshape(b, c, tt, V)
